# revision 8
# baseline (speedup 1.0000x reference)
"""BSNN (block-sparse MLP with sine activations) forward on 8 TRN2 NeuronCores.

Network (per point x in R^3):
  A1 = sin(x W0 + b0)          3 -> 64
  A2 = sin(A1 W1 + b1)         64 -> 128
  A3 = sin(A2 (W2*m2) + b2)    128 -> 256   2 blocks of (64 -> 128)
  A4 = sin(A3 (W3*m3) + b3)    256 -> 512   4 blocks
  A5 = sin(A4 (W4*m4) + b4)    512 -> 1024  8 blocks
  out = A5 W5 + b5             1024 -> 1

Data-parallel: X sharded over 8 cores (16384 points each), weights replicated.
On-chip layout: activations transposed (channels on SBUF partitions, points on
the free dim).  Block matmuls (K=64, M=128) are row-packed in pairs via
tile_position so two blocks run concurrently in the PE array.  Sin runs on
ScalarE fused with the PSUM->SBUF drain and the per-channel bias add; ScalarE
is the throughput bottleneck (~2e3 sins/point), everything else hides under it.
"""

import os
import sys

for _p in ("/opt/trn_rl_repo",):
    if _p not in sys.path and os.path.isdir(_p):
        sys.path.insert(0, _p)

import numpy as np

import concourse.bass as bass
import concourse.bacc as bacc
import concourse.mybir as mybir
import concourse.tile as tile
from concourse.bass_utils import run_bass_kernel_spmd

F32 = mybir.dt.float32
F32R = mybir.dt.float32r
SIN = mybir.ActivationFunctionType.Sin

N_CORES = 8
N_TOTAL = 131072
N_CORE = N_TOTAL // N_CORES  # 16384
CHUNK = 2048                 # points per outer chunk
UNIT = 512                   # points per matmul (one PSUM bank of fp32)
HALF = CHUNK // 2

# matmul operand dtype: float32r streams 1 row/cycle (vs 4 for float32) at
# free-dim >= 256.  Flip to F32 if precision ever becomes a problem.
MM_DT = F32R


def _r(ap):
    """Activations are already stored as MM_DT; pass through."""
    return ap


def _build_nc(with_bias=True):
    nc = bacc.Bacc(None, target_bir_lowering=False, debug=False)

    X = nc.declare_dram_parameter("X", [N_CORE, 3], F32, isOutput=False)
    # host-side prepacked weights (see _pack_weights)
    w0d = nc.declare_dram_parameter("w0p", [3, 64], F32, isOutput=False)
    w1d = nc.declare_dram_parameter("w1p", [128, 128], F32, isOutput=False)
    w2d = nc.declare_dram_parameter("w2p", [128, 128], F32, isOutput=False)
    w3d = nc.declare_dram_parameter("w3p", [2 * 128, 128], F32, isOutput=False)
    w4d = nc.declare_dram_parameter("w4p", [4 * 128, 128], F32, isOutput=False)
    w5d = nc.declare_dram_parameter("w5p", [128, 8], F32, isOutput=False)
    bd = nc.declare_dram_parameter("bp", [128, 16], F32, isOutput=False)
    b5d = nc.declare_dram_parameter("b5p", [1, 1], F32, isOutput=False)
    OUT = nc.declare_dram_parameter("out", [N_CORE, 1], F32, isOutput=True)

    with tile.TileContext(nc) as tc:
        with (
            tc.tile_pool(name="wp", bufs=1) as wp,
            tc.tile_pool(name="xp", bufs=3) as xp,
            tc.tile_pool(name="a1p", bufs=2) as a1p,
            tc.tile_pool(name="a2p", bufs=3) as a2p,
            tc.tile_pool(name="a3p", bufs=6) as a3p,
            tc.tile_pool(name="a4p", bufs=10) as a4p,
            tc.tile_pool(name="a5p", bufs=6) as a5p,
            tc.tile_pool(name="op", bufs=2) as op,
            tc.tile_pool(name="pp", bufs=3, space="PSUM") as pp,
            tc.tile_pool(name="p5", bufs=2, space="PSUM") as p5,
        ):
            # --- resident weights/biases -------------------------------
            w0 = wp.tile([3, 64], F32)
            nc.sync.dma_start(out=w0[:], in_=w0d[:])
            w1 = wp.tile([128, 128], F32)
            nc.sync.dma_start(out=w1[:], in_=w1d[:])
            w2 = wp.tile([128, 128], F32)
            nc.sync.dma_start(out=w2[:], in_=w2d[:])
            w3 = [wp.tile([128, 128], F32, tag=f"w3_{t}", name=f"w3_{t}") for t in range(2)]
            for t in range(2):
                nc.sync.dma_start(out=w3[t][:], in_=w3d[128 * t:128 * (t + 1), :])
            w4 = [wp.tile([128, 128], F32, tag=f"w4_{t}", name=f"w4_{t}") for t in range(4)]
            for t in range(4):
                nc.sync.dma_start(out=w4[t][:], in_=w4d[128 * t:128 * (t + 1), :])
            w5 = wp.tile([128, 8], F32)
            nc.sync.dma_start(out=w5[:], in_=w5d[:])
            # bias columns: 0: b0 stacked twice; 1: b1; 2-3: b2; 4-7: b3; 8-15: b4
            bt = wp.tile([128, 16], F32)
            nc.sync.dma_start(out=bt[:], in_=bd[:])

            # rounded (float32r) weight copies for full-rate PE streaming.
            # All matmul operands must be produced on ONE engine's semaphore
            # (self-loading fp32r matmuls tolerate only a single sync wait):
            # ScalarE rounds w1..w5, VectorE rounds w0 + X^T chunks.
            CP = mybir.ActivationFunctionType.Copy
            w0r = wp.tile([3, 64], MM_DT)
            nc.vector.tensor_copy(w0r[:], w0[:])
            w1r = wp.tile([128, 128], MM_DT)
            nc.scalar.activation(w1r[:], w1[:], CP)
            w2r = wp.tile([128, 128], MM_DT)
            nc.scalar.activation(w2r[:], w2[:], CP)
            w3r = [wp.tile([128, 128], MM_DT, tag=f"w3r_{t}", name=f"w3r_{t}")
                   for t in range(2)]
            for t in range(2):
                nc.scalar.activation(w3r[t][:], w3[t][:], CP)
            w4r = [wp.tile([128, 128], MM_DT, tag=f"w4r_{t}", name=f"w4r_{t}")
                   for t in range(4)]
            for t in range(4):
                nc.scalar.activation(w4r[t][:], w4[t][:], CP)
            w5r = wp.tile([128, 8], MM_DT)
            nc.scalar.activation(w5r[:], w5[:], CP)

            B0 = bt[:, 0:1]
            B1 = bt[:, 1:2]
            B2 = [bt[:, 2 + g:3 + g] for g in range(2)]
            B3 = [bt[:, 4 + g:5 + g] for g in range(4)]
            B4 = [bt[:, 8 + g:9 + g] for g in range(8)]

            n_chunks = N_CORE // CHUNK
            for k in range(n_chunks):
                r0 = k * CHUNK
                # X^T chunk: [3, CHUNK] (strided DMA; channels on partitions)
                xt = xp.tile([3, CHUNK], F32)
                nc.sync.dma_start(out=xt[:], in_=X[r0:r0 + CHUNK, :].transpose([1, 0]))
                xtr = xp.tile([3, CHUNK], MM_DT, name="xtr")
                nc.vector.tensor_copy(xtr[:], xt[:])

                # ---- L0: 3 -> 64, stacked halves -> A1 [128, HALF] ----
                # partitions 0-63: points [0, HALF) ; 64-127: [HALF, CHUNK)
                # (col-tiling is not legal for self-loading fp32r weights, so
                #  the upper half is stacked at the ACT drain, not in PSUM)
                psa = pp.tile([64, HALF], F32, tag="ps", name="psa")
                psb = pp.tile([64, HALF], F32, tag="ps", name="psb")
                for j in range(HALF // UNIT):
                    c = j * UNIT
                    nc.tensor.matmul(
                        out=psa[:, c:c + UNIT], lhsT=w0r[:],
                        rhs=xtr[:, c:c + UNIT], start=True, stop=True)
                    nc.tensor.matmul(
                        out=psb[:, c:c + UNIT], lhsT=w0r[:],
                        rhs=xtr[:, HALF + c:HALF + c + UNIT],
                        start=True, stop=True)
                a1 = a1p.tile([128, HALF], MM_DT)
                if with_bias:
                    nc.scalar.activation(a1[0:64, :], psa[:], SIN, bias=B0[0:64])
                    nc.scalar.activation(a1[64:128, :], psb[:], SIN,
                                         bias=B0[64:128])
                else:
                    nc.scalar.activation(a1[0:64, :], psa[:], SIN)
                    nc.scalar.activation(a1[64:128, :], psb[:], SIN)

                # ---- L1: 64 -> 128 (row-packed halves) -> A2 ---------
                # a2[s] tile holds point-blocks {s, s+2} (cols 0:UNIT, UNIT:2U)
                a2 = []
                for j in range(HALF // UNIT):
                    c = j * UNIT
                    ps = pp.tile([128, 2 * UNIT], F32, tag="ps", name="ps")
                    nc.tensor.matmul(
                        out=ps[:, 0:UNIT], lhsT=(w1r[0:64, :]),
                        rhs=_r(a1[0:64, c:c + UNIT]), start=True, stop=True)
                    nc.tensor.matmul(
                        out=ps[:, UNIT:2 * UNIT], lhsT=(w1r[64:128, :]),
                        rhs=_r(a1[64:128, c:c + UNIT]), start=True, stop=True)
                    t = a2p.tile([128, 2 * UNIT], MM_DT, name="a2t")
                    if with_bias:
                        nc.scalar.activation(t[:], ps[:], SIN, bias=B1)
                    else:
                        nc.scalar.activation(t[:], ps[:], SIN)
                    a2.append(t)

                def a2u(p):  # A2 unit for point-block p (128 ch x UNIT)
                    return a2[p % 2][:, (p // 2) * UNIT:(p // 2 + 1) * UNIT]

                n_pb = CHUNK // UNIT  # point-blocks per chunk

                # ---- L2: 2 blocks 64->128 -> A3 (256 ch = pair tile) --
                a3 = []  # [p] -> [128, 2U] tile: cols 0:U ch 0-127, U:2U ch 128-255
                for p in range(n_pb):
                    src = a2u(p)
                    ps = pp.tile([128, 2 * UNIT], F32, tag="ps", name="ps")
                    nc.tensor.matmul(
                        out=ps[:, 0:UNIT], lhsT=(w2r[0:64, :]),
                        rhs=_r(src[0:64, :]), start=True, stop=True)
                    nc.tensor.matmul(
                        out=ps[:, UNIT:2 * UNIT], lhsT=(w2r[64:128, :]),
                        rhs=_r(src[64:128, :]), start=True, stop=True)
                    t = a3p.tile([128, 2 * UNIT], MM_DT, name="a3t")
                    if with_bias:
                        nc.scalar.activation(t[:, 0:UNIT], ps[:, 0:UNIT], SIN,
                                             bias=B2[0])
                        nc.scalar.activation(t[:, UNIT:2 * UNIT],
                                             ps[:, UNIT:2 * UNIT], SIN, bias=B2[1])
                    else:
                        nc.scalar.activation(t[:], ps[:], SIN)
                    a3.append(t)

                # ---- L3: 4 blocks -> A4 (512 ch = 2 pair tiles) -------
                a4 = []  # [p][q] q in 0,1: cols 0:U ch 256q.., U:2U ch 256q+128..
                for p in range(n_pb):
                    row = []
                    for q in range(2):  # block pair (2q, 2q+1)
                        src = a3[p][:, q * UNIT:(q + 1) * UNIT]
                        ps = pp.tile([128, 2 * UNIT], F32, tag="ps", name="ps")
                        nc.tensor.matmul(
                            out=ps[:, 0:UNIT], lhsT=(w3r[q][0:64, :]),
                            rhs=_r(src[0:64, :]), start=True, stop=True)
                        nc.tensor.matmul(
                            out=ps[:, UNIT:2 * UNIT], lhsT=(w3r[q][64:128, :]),
                            rhs=_r(src[64:128, :]), start=True, stop=True)
                        t = a4p.tile([128, 2 * UNIT], MM_DT, name="a4t")
                        if with_bias:
                            nc.scalar.activation(t[:, 0:UNIT], ps[:, 0:UNIT], SIN,
                                                 bias=B3[2 * q])
                            nc.scalar.activation(t[:, UNIT:2 * UNIT],
                                                 ps[:, UNIT:2 * UNIT], SIN,
                                                 bias=B3[2 * q + 1])
                        else:
                            nc.scalar.activation(t[:], ps[:], SIN)
                        row.append(t)
                    a4.append(row)

                # ---- L4 + L5 fused per point-block --------------------
                for p in range(n_pb):
                    o_ps = p5.tile([1, UNIT], F32, tag="o", name="ops")
                    for q in range(4):  # block pair (2q, 2q+1) of L4
                        src = a4[p][q // 2][:, (q % 2) * UNIT:(q % 2 + 1) * UNIT]
                        ps = pp.tile([128, 2 * UNIT], F32, tag="ps", name="ps")
                        nc.tensor.matmul(
                            out=ps[:, 0:UNIT], lhsT=(w4r[q][0:64, :]),
                            rhs=_r(src[0:64, :]), start=True, stop=True)
                        nc.tensor.matmul(
                            out=ps[:, UNIT:2 * UNIT], lhsT=(w4r[q][64:128, :]),
                            rhs=_r(src[64:128, :]), start=True, stop=True)
                        t = a5p.tile([128, 2 * UNIT], MM_DT, name="a5t")
                        if with_bias:
                            nc.scalar.activation(t[:, 0:UNIT], ps[:, 0:UNIT], SIN,
                                                 bias=B4[2 * q])
                            nc.scalar.activation(t[:, UNIT:2 * UNIT],
                                                 ps[:, UNIT:2 * UNIT], SIN,
                                                 bias=B4[2 * q + 1])
                        else:
                            nc.scalar.activation(t[:], ps[:], SIN)
                        # L5: accumulate the two fresh 128-ch groups
                        nc.tensor.matmul(
                            out=o_ps[:], lhsT=(w5r[:, 2 * q:2 * q + 1]),
                            rhs=_r(t[:, 0:UNIT]), start=(q == 0), stop=False)
                        nc.tensor.matmul(
                            out=o_ps[:], lhsT=(w5r[:, 2 * q + 1:2 * q + 2]),
                            rhs=_r(t[:, UNIT:2 * UNIT]), start=False,
                            stop=(q == 3))
                    o_sb = op.tile([1, UNIT], F32, tag="osb", name="osb")
                    nc.vector.tensor_copy(o_sb[:], o_ps[:])
                    nc.sync.dma_start(
                        out=OUT.transpose([1, 0])[0:1, r0 + p * UNIT:
                                                  r0 + (p + 1) * UNIT],
                        in_=o_sb[:])
    nc.compile()
    return nc


def _pack_weights(inputs):
    W = {l: np.asarray(inputs[f"W{l}"], np.float32) for l in range(6)}
    b = {l: np.asarray(inputs[f"b{l}"], np.float32) for l in range(6)}
    w0p = W[0]                                            # [3, 64]
    w1p = np.concatenate([W[1], W[1]], axis=0)            # [128, 128]
    w2p = np.concatenate(
        [W[2][0:64, 0:128], W[2][64:128, 128:256]], axis=0)

    def blocks(Wl, nb):
        return [Wl[64 * i:64 * (i + 1), 128 * i:128 * (i + 1)] for i in range(nb)]

    w3p = np.concatenate(blocks(W[3], 4), axis=0)         # [256, 128]
    w4p = np.concatenate(blocks(W[4], 8), axis=0)         # [512, 128]
    w5p = np.ascontiguousarray(W[5].reshape(8, 128).T)    # [128, 8]
    bp = np.zeros((128, 16), np.float32)
    bp[0:64, 0] = b[0][0]
    bp[64:128, 0] = b[0][0]
    bp[:, 1] = b[1][0]
    for g in range(2):
        bp[:, 2 + g] = b[2][0, 128 * g:128 * (g + 1)]
    for g in range(4):
        bp[:, 4 + g] = b[3][0, 128 * g:128 * (g + 1)]
    for g in range(8):
        bp[:, 8 + g] = b[4][0, 128 * g:128 * (g + 1)]
    b5p = b[5].reshape(1, 1)
    return dict(w0p=w0p, w1p=np.ascontiguousarray(w1p),
                w2p=np.ascontiguousarray(w2p), w3p=np.ascontiguousarray(w3p),
                w4p=np.ascontiguousarray(w4p), w5p=w5p,
                bp=bp, b5p=np.ascontiguousarray(b5p))


_NC_CACHE = {}


def _get_nc(with_bias=True):
    if with_bias not in _NC_CACHE:
        _NC_CACHE[with_bias] = _build_nc(with_bias)
    return _NC_CACHE[with_bias]


def kernel(**inputs):
    zero_bias = all(
        not np.any(np.asarray(inputs[f"b{l}"], np.float32)) for l in range(5))
    nc = _get_nc(with_bias=not zero_bias)
    X = np.ascontiguousarray(np.asarray(inputs["X"], np.float32))
    packed = _pack_weights(inputs)
    in_maps = []
    for i in range(N_CORES):
        m = {"X": np.ascontiguousarray(X[i * N_CORE:(i + 1) * N_CORE])}
        m.update(packed)
        in_maps.append(m)
    res = run_bass_kernel_spmd(nc, in_maps, core_ids=list(range(N_CORES)))
    out = np.concatenate([r["out"] for r in res.results], axis=0)
    out = out + np.asarray(inputs["b5"], np.float32).reshape(1, 1)
    return out.astype(np.float32)


if __name__ == "__main__":
    nc = _build_nc()
    print("build ok")


# revision 10
# speedup vs baseline: 211.3786x; 211.3786x over previous
"""BSNN (block-sparse MLP with sine activations) forward on 8 TRN2 NeuronCores.

Network (per point x in R^3):
  A1 = sin(x W0 + b0)          3 -> 64
  A2 = sin(A1 W1 + b1)         64 -> 128
  A3 = sin(A2 (W2*m2) + b2)    128 -> 256   2 blocks of (64 -> 128)
  A4 = sin(A3 (W3*m3) + b3)    256 -> 512   4 blocks
  A5 = sin(A4 (W4*m4) + b4)    512 -> 1024  8 blocks
  out = A5 W5 + b5             1024 -> 1

Data-parallel: X sharded over 8 cores (16384 points each), weights replicated.
On-chip layout: activations transposed (channels on SBUF partitions, points on
the free dim).  Block matmuls (K=64, M=128) are row-packed in pairs via
tile_position so two blocks run concurrently in the PE array.  Sin runs on
ScalarE fused with the PSUM->SBUF drain and the per-channel bias add; ScalarE
is the throughput bottleneck (~2e3 sins/point), everything else hides under it.
"""

import os
import sys

for _p in ("/opt/trn_rl_repo",):
    if _p not in sys.path and os.path.isdir(_p):
        sys.path.insert(0, _p)

import numpy as np

import concourse.bass as bass
import concourse.bacc as bacc
import concourse.mybir as mybir
import concourse.tile as tile
from concourse.bass_utils import run_bass_kernel_spmd

F32 = mybir.dt.float32
F32R = mybir.dt.float32r
SIN = mybir.ActivationFunctionType.Sin

N_CORES = 8
N_TOTAL = 131072
N_CORE = N_TOTAL // N_CORES  # 16384
CHUNK = 2048                 # points per outer chunk
UNIT = 512                   # points per matmul (one PSUM bank of fp32)
HALF = CHUNK // 2

# matmul operand dtype: float32r streams 1 row/cycle (vs 4 for float32) at
# free-dim >= 256.  Flip to F32 if precision ever becomes a problem.
MM_DT = F32R


def _r(ap):
    """Activations are already stored as MM_DT; pass through."""
    return ap


def _build_nc(with_bias=True, repeat=1):
    nc = bacc.Bacc(None, target_bir_lowering=False, debug=False)

    XT = nc.declare_dram_parameter("Xt", [3, N_CORE], F32, isOutput=False)
    # host-side prepacked weights (see _pack_weights)
    w0d = nc.declare_dram_parameter("w0p", [3, 64], F32, isOutput=False)
    w1d = nc.declare_dram_parameter("w1p", [128, 128], F32, isOutput=False)
    w2d = nc.declare_dram_parameter("w2p", [128, 128], F32, isOutput=False)
    w3d = nc.declare_dram_parameter("w3p", [2 * 128, 128], F32, isOutput=False)
    w4d = nc.declare_dram_parameter("w4p", [4 * 128, 128], F32, isOutput=False)
    w5d = nc.declare_dram_parameter("w5p", [128, 8], F32, isOutput=False)
    bd = nc.declare_dram_parameter("bp", [128, 16], F32, isOutput=False)
    b5d = nc.declare_dram_parameter("b5p", [1, 1], F32, isOutput=False)
    OUT = nc.declare_dram_parameter("out", [N_CORE, 1], F32, isOutput=True)

    with tile.TileContext(nc) as tc:
        with (
            tc.tile_pool(name="wp", bufs=1) as wp,
            tc.tile_pool(name="xp", bufs=3) as xp,
            tc.tile_pool(name="a1p", bufs=2) as a1p,
            tc.tile_pool(name="a2p", bufs=3) as a2p,
            tc.tile_pool(name="a3p", bufs=6) as a3p,
            tc.tile_pool(name="a4p", bufs=10) as a4p,
            tc.tile_pool(name="a5p", bufs=6) as a5p,
            tc.tile_pool(name="op", bufs=2) as op,
            tc.tile_pool(name="pp", bufs=3, space="PSUM") as pp,
            tc.tile_pool(name="p5", bufs=2, space="PSUM") as p5,
        ):
            # --- resident weights/biases -------------------------------
            w0 = wp.tile([3, 64], F32)
            nc.sync.dma_start(out=w0[:], in_=w0d[:])
            w1 = wp.tile([128, 128], F32)
            nc.sync.dma_start(out=w1[:], in_=w1d[:])
            w2 = wp.tile([128, 128], F32)
            nc.sync.dma_start(out=w2[:], in_=w2d[:])
            w3 = [wp.tile([128, 128], F32, tag=f"w3_{t}", name=f"w3_{t}") for t in range(2)]
            for t in range(2):
                nc.sync.dma_start(out=w3[t][:], in_=w3d[128 * t:128 * (t + 1), :])
            w4 = [wp.tile([128, 128], F32, tag=f"w4_{t}", name=f"w4_{t}") for t in range(4)]
            for t in range(4):
                nc.sync.dma_start(out=w4[t][:], in_=w4d[128 * t:128 * (t + 1), :])
            w5 = wp.tile([128, 8], F32)
            nc.sync.dma_start(out=w5[:], in_=w5d[:])
            # bias columns: 0: b0 stacked twice; 1: b1; 2-3: b2; 4-7: b3; 8-15: b4
            bt = wp.tile([128, 16], F32)
            nc.sync.dma_start(out=bt[:], in_=bd[:])

            # rounded (float32r) weight copies for full-rate PE streaming.
            # All matmul operands must be produced on ONE engine's semaphore
            # (self-loading fp32r matmuls tolerate only a single sync wait):
            # ScalarE rounds w1..w5, VectorE rounds w0 + X^T chunks.
            CP = mybir.ActivationFunctionType.Copy
            w0r = wp.tile([3, 64], MM_DT)
            nc.vector.tensor_copy(w0r[:], w0[:])
            w1r = wp.tile([128, 128], MM_DT)
            nc.scalar.activation(w1r[:], w1[:], CP)
            w2r = wp.tile([128, 128], MM_DT)
            nc.scalar.activation(w2r[:], w2[:], CP)
            w3r = [wp.tile([128, 128], MM_DT, tag=f"w3r_{t}", name=f"w3r_{t}")
                   for t in range(2)]
            for t in range(2):
                nc.scalar.activation(w3r[t][:], w3[t][:], CP)
            w4r = [wp.tile([128, 128], MM_DT, tag=f"w4r_{t}", name=f"w4r_{t}")
                   for t in range(4)]
            for t in range(4):
                nc.scalar.activation(w4r[t][:], w4[t][:], CP)
            w5r = wp.tile([128, 8], MM_DT)
            nc.scalar.activation(w5r[:], w5[:], CP)

            B0 = bt[:, 0:1]
            B1 = bt[:, 1:2]
            B2 = [bt[:, 2 + g:3 + g] for g in range(2)]
            B3 = [bt[:, 4 + g:5 + g] for g in range(4)]
            B4 = [bt[:, 8 + g:9 + g] for g in range(8)]

            n_chunks = N_CORE // CHUNK
            for k_rep in range(repeat * n_chunks):
                k = k_rep % n_chunks
                r0 = k * CHUNK
                # X^T chunk: [3, CHUNK] (strided DMA; channels on partitions)
                xt = xp.tile([3, CHUNK], F32)
                nc.sync.dma_start(out=xt[:], in_=XT[:, r0:r0 + CHUNK])
                xtr = xp.tile([3, CHUNK], MM_DT, name="xtr")
                nc.vector.tensor_copy(xtr[:], xt[:])

                # ---- L0: 3 -> 64, stacked halves -> A1 [128, HALF] ----
                # partitions 0-63: points [0, HALF) ; 64-127: [HALF, CHUNK)
                # (col-tiling is not legal for self-loading fp32r weights, so
                #  the upper half is stacked at the ACT drain, not in PSUM)
                psa = pp.tile([64, HALF], F32, tag="ps", name="psa")
                psb = pp.tile([64, HALF], F32, tag="ps", name="psb")
                for j in range(HALF // UNIT):
                    c = j * UNIT
                    nc.tensor.matmul(
                        out=psa[:, c:c + UNIT], lhsT=w0r[:],
                        rhs=xtr[:, c:c + UNIT], start=True, stop=True)
                    nc.tensor.matmul(
                        out=psb[:, c:c + UNIT], lhsT=w0r[:],
                        rhs=xtr[:, HALF + c:HALF + c + UNIT],
                        start=True, stop=True)
                a1 = a1p.tile([128, HALF], MM_DT)
                if with_bias:
                    nc.scalar.activation(a1[0:64, :], psa[:], SIN, bias=B0[0:64])
                    nc.scalar.activation(a1[64:128, :], psb[:], SIN,
                                         bias=B0[64:128])
                else:
                    nc.scalar.activation(a1[0:64, :], psa[:], SIN)
                    nc.scalar.activation(a1[64:128, :], psb[:], SIN)

                # ---- L1: 64 -> 128 (row-packed halves) -> A2 ---------
                # a2[s] tile holds point-blocks {s, s+2} (cols 0:UNIT, UNIT:2U)
                a2 = []
                for j in range(HALF // UNIT):
                    c = j * UNIT
                    ps = pp.tile([128, 2 * UNIT], F32, tag="ps", name="ps")
                    nc.tensor.matmul(
                        out=ps[:, 0:UNIT], lhsT=(w1r[0:64, :]),
                        rhs=_r(a1[0:64, c:c + UNIT]), start=True, stop=True)
                    nc.tensor.matmul(
                        out=ps[:, UNIT:2 * UNIT], lhsT=(w1r[64:128, :]),
                        rhs=_r(a1[64:128, c:c + UNIT]), start=True, stop=True)
                    t = a2p.tile([128, 2 * UNIT], MM_DT, name="a2t")
                    if with_bias:
                        nc.scalar.activation(t[:], ps[:], SIN, bias=B1)
                    else:
                        nc.scalar.activation(t[:], ps[:], SIN)
                    a2.append(t)

                def a2u(p):  # A2 unit for point-block p (128 ch x UNIT)
                    return a2[p % 2][:, (p // 2) * UNIT:(p // 2 + 1) * UNIT]

                n_pb = CHUNK // UNIT  # point-blocks per chunk

                # ---- L2: 2 blocks 64->128 -> A3 (256 ch = pair tile) --
                a3 = []  # [p] -> [128, 2U] tile: cols 0:U ch 0-127, U:2U ch 128-255
                for p in range(n_pb):
                    src = a2u(p)
                    ps = pp.tile([128, 2 * UNIT], F32, tag="ps", name="ps")
                    nc.tensor.matmul(
                        out=ps[:, 0:UNIT], lhsT=(w2r[0:64, :]),
                        rhs=_r(src[0:64, :]), start=True, stop=True)
                    nc.tensor.matmul(
                        out=ps[:, UNIT:2 * UNIT], lhsT=(w2r[64:128, :]),
                        rhs=_r(src[64:128, :]), start=True, stop=True)
                    t = a3p.tile([128, 2 * UNIT], MM_DT, name="a3t")
                    if with_bias:
                        nc.scalar.activation(t[:, 0:UNIT], ps[:, 0:UNIT], SIN,
                                             bias=B2[0])
                        nc.scalar.activation(t[:, UNIT:2 * UNIT],
                                             ps[:, UNIT:2 * UNIT], SIN, bias=B2[1])
                    else:
                        nc.scalar.activation(t[:], ps[:], SIN)
                    a3.append(t)

                # ---- L3: 4 blocks -> A4 (512 ch = 2 pair tiles) -------
                a4 = []  # [p][q] q in 0,1: cols 0:U ch 256q.., U:2U ch 256q+128..
                for p in range(n_pb):
                    row = []
                    for q in range(2):  # block pair (2q, 2q+1)
                        src = a3[p][:, q * UNIT:(q + 1) * UNIT]
                        ps = pp.tile([128, 2 * UNIT], F32, tag="ps", name="ps")
                        nc.tensor.matmul(
                            out=ps[:, 0:UNIT], lhsT=(w3r[q][0:64, :]),
                            rhs=_r(src[0:64, :]), start=True, stop=True)
                        nc.tensor.matmul(
                            out=ps[:, UNIT:2 * UNIT], lhsT=(w3r[q][64:128, :]),
                            rhs=_r(src[64:128, :]), start=True, stop=True)
                        t = a4p.tile([128, 2 * UNIT], MM_DT, name="a4t")
                        if with_bias:
                            nc.scalar.activation(t[:, 0:UNIT], ps[:, 0:UNIT], SIN,
                                                 bias=B3[2 * q])
                            nc.scalar.activation(t[:, UNIT:2 * UNIT],
                                                 ps[:, UNIT:2 * UNIT], SIN,
                                                 bias=B3[2 * q + 1])
                        else:
                            nc.scalar.activation(t[:], ps[:], SIN)
                        row.append(t)
                    a4.append(row)

                # ---- L4 + L5 fused per point-block --------------------
                for p in range(n_pb):
                    o_ps = p5.tile([1, UNIT], F32, tag="o", name="ops")
                    for q in range(4):  # block pair (2q, 2q+1) of L4
                        src = a4[p][q // 2][:, (q % 2) * UNIT:(q % 2 + 1) * UNIT]
                        ps = pp.tile([128, 2 * UNIT], F32, tag="ps", name="ps")
                        nc.tensor.matmul(
                            out=ps[:, 0:UNIT], lhsT=(w4r[q][0:64, :]),
                            rhs=_r(src[0:64, :]), start=True, stop=True)
                        nc.tensor.matmul(
                            out=ps[:, UNIT:2 * UNIT], lhsT=(w4r[q][64:128, :]),
                            rhs=_r(src[64:128, :]), start=True, stop=True)
                        t = a5p.tile([128, 2 * UNIT], MM_DT, name="a5t")
                        if with_bias:
                            nc.scalar.activation(t[:, 0:UNIT], ps[:, 0:UNIT], SIN,
                                                 bias=B4[2 * q])
                            nc.scalar.activation(t[:, UNIT:2 * UNIT],
                                                 ps[:, UNIT:2 * UNIT], SIN,
                                                 bias=B4[2 * q + 1])
                        else:
                            nc.scalar.activation(t[:], ps[:], SIN)
                        # L5: accumulate the two fresh 128-ch groups
                        nc.tensor.matmul(
                            out=o_ps[:], lhsT=(w5r[:, 2 * q:2 * q + 1]),
                            rhs=_r(t[:, 0:UNIT]), start=(q == 0), stop=False)
                        nc.tensor.matmul(
                            out=o_ps[:], lhsT=(w5r[:, 2 * q + 1:2 * q + 2]),
                            rhs=_r(t[:, UNIT:2 * UNIT]), start=False,
                            stop=(q == 3))
                    o_sb = op.tile([1, UNIT], F32, tag="osb", name="osb")
                    nc.vector.tensor_copy(o_sb[:], o_ps[:])
                    nc.sync.dma_start(
                        out=OUT.transpose([1, 0])[0:1, r0 + p * UNIT:
                                                  r0 + (p + 1) * UNIT],
                        in_=o_sb[:])
    nc.compile()
    return nc


def _pack_weights(inputs):
    W = {l: np.asarray(inputs[f"W{l}"], np.float32) for l in range(6)}
    b = {l: np.asarray(inputs[f"b{l}"], np.float32) for l in range(6)}
    w0p = W[0]                                            # [3, 64]
    w1p = np.concatenate([W[1], W[1]], axis=0)            # [128, 128]
    w2p = np.concatenate(
        [W[2][0:64, 0:128], W[2][64:128, 128:256]], axis=0)

    def blocks(Wl, nb):
        return [Wl[64 * i:64 * (i + 1), 128 * i:128 * (i + 1)] for i in range(nb)]

    w3p = np.concatenate(blocks(W[3], 4), axis=0)         # [256, 128]
    w4p = np.concatenate(blocks(W[4], 8), axis=0)         # [512, 128]
    w5p = np.ascontiguousarray(W[5].reshape(8, 128).T)    # [128, 8]
    bp = np.zeros((128, 16), np.float32)
    bp[0:64, 0] = b[0][0]
    bp[64:128, 0] = b[0][0]
    bp[:, 1] = b[1][0]
    for g in range(2):
        bp[:, 2 + g] = b[2][0, 128 * g:128 * (g + 1)]
    for g in range(4):
        bp[:, 4 + g] = b[3][0, 128 * g:128 * (g + 1)]
    for g in range(8):
        bp[:, 8 + g] = b[4][0, 128 * g:128 * (g + 1)]
    b5p = b[5].reshape(1, 1)
    return dict(w0p=w0p, w1p=np.ascontiguousarray(w1p),
                w2p=np.ascontiguousarray(w2p), w3p=np.ascontiguousarray(w3p),
                w4p=np.ascontiguousarray(w4p), w5p=w5p,
                bp=bp, b5p=np.ascontiguousarray(b5p))


_NC_CACHE = {}


def _get_nc(with_bias=True, repeat=1):
    key = (with_bias, repeat)
    if key not in _NC_CACHE:
        _NC_CACHE[key] = _build_nc(with_bias, repeat)
    return _NC_CACHE[key]


def _make_in_maps(inputs):
    X = np.asarray(inputs["X"], np.float32)
    packed = _pack_weights(inputs)
    in_maps = []
    for i in range(N_CORES):
        xs = X[i * N_CORE:(i + 1) * N_CORE]
        m = {"Xt": np.ascontiguousarray(xs.T)}
        m.update(packed)
        in_maps.append(m)
    return in_maps


def kernel(**inputs):
    zero_bias = all(
        not np.any(np.asarray(inputs[f"b{l}"], np.float32)) for l in range(5))
    nc = _get_nc(with_bias=not zero_bias)
    in_maps = _make_in_maps(inputs)
    res = run_bass_kernel_spmd(nc, in_maps, core_ids=list(range(N_CORES)))
    out = np.concatenate([r["out"] for r in res.results], axis=0)
    out = out + np.asarray(inputs["b5"], np.float32).reshape(1, 1)
    return out.astype(np.float32)


if __name__ == "__main__":
    nc = _build_nc()
    print("build ok")
